# revision 21
# baseline (speedup 1.0000x reference)
"""Trainium2 Bass kernel for DifferentiableMemory (B=8, S=4096, H=1024, M=1024).

Data-parallel over batch: one batch per NeuronCore x 8 cores, weights
replicated. All on-device activations live feature-on-partition ("T layout"),
produced by free host-side transposes, so no on-device transposes are needed:

  per core (batch b), with xT = hidden[b].T as [H, S]:
    KmemT = WkT-proj(initT)     initT = xT[:, idx] (static gather, host-side)
    kn    = KmemT / colnorm(KmemT)   (Square + PE ones-reduce over partitions)
    V2    = init @ Wvo^T        Wvo = Wo2 @ Wv host-precomputed: folds
                                (attn @ Vmem) @ Wo2^T into attn @ V2
    per 512-col chunk of s:
      QT    = WqT-proj(xT chunk); qn = QT / colnorm(QT)
      simT  = kn^T-matmul(qn)    [m, s]  (cosine sims, bounded -> exp w/o max)
      expT  = Exp(simT); attnT = expT * recip(ones-reduce(expT))
      outT  = Wo1-proj(xT chunk) + V2-matmul(attnT) + bo

In the default fp8 mode, the Q/K/V2/sim/attn-V2 matmuls run as fp8e4
DoubleRow (2 contraction rows per PE cell, 2x throughput); Wo1 @ x stays
bf16 and is accumulated in a separate PSUM chain, combined by DVE. fp8
operands are pre-scaled into e4m3's normal range (weights x32, qn/kn x32,
attn x256); every compensation folds into an existing ACT scale parameter,
and the x32 on Q/K cancels inside the L2 normalization. Row-scale
broadcasts ([1,N] -> [128,N]) are a K=1 PE matmul against a ones row
(this walrus build rejects InstPartitionBroadcast).
"""
import sys

sys.path.insert(0, "/opt/trn_rl_repo")

import numpy as np
import ml_dtypes

BF16 = ml_dtypes.bfloat16
FP8 = ml_dtypes.float8_e4m3fn

B, S, H, M = 8, 4096, 1024, 1024
N_CORES = 8
P = 128          # partitions
KT = H // P      # 8 feature tiles
SC = 512         # s-chunk (PSUM bank = 512 fp32)
NCH = S // SC    # 8 chunks
MT = M // P      # 8 memory tiles
W8SCALE = 32.0   # fp8 pre-scale for Wq/Wk/Wvo (their entries are ~1/32)
ASCALE = 256.0   # fp8 pre-scale for softmax weights (~1/1024 each)

_cache = {}


def _idx():
    """Replicate reference: jnp.linspace(0.0, s-1, M).astype(int32).
    Computed with in-process jax so platform-specific fp32 rounding matches
    the grader's reference; numpy fallback differs in at most a few slots."""
    if "idx" in _cache:
        return _cache["idx"]
    try:
        import jax.numpy as jnp

        idx = np.asarray(jnp.linspace(0.0, S - 1, M).astype(jnp.int32))
    except Exception:
        idx = np.linspace(0.0, S - 1, M).astype(np.float32).astype(np.int32)
    _cache["idx"] = idx
    return idx


def _split_excess_waits(nc, mybir):
    """This container's walrus accepts at most 1 sem-wait per instruction
    (setupSyncWait raises "Too many sync wait commands" beyond that), while
    Tile's add_semaphores freely attaches several. Move excess waits onto
    preceding same-engine NoOps - engine streams execute in order, so a wait
    on an earlier instruction gates everything after it."""
    n = 0
    for f in nc.m.functions:
        for bb in f.blocks:
            insts = list(bb.instructions)
            new = []
            changed = False
            for inst in insts:
                si = getattr(inst, "sync_info", None)
                waits = list(si.on_wait) if si is not None and si.on_wait else []
                if len(waits) > 1:
                    for w in waits[:-1]:
                        new.append(
                            mybir.InstNoOp(
                                name=f"{inst.name}-wsplit{n}",
                                engine=inst.engine,
                                sync_info=mybir.SyncInfo(on_wait=[w], on_update=[]),
                                bass_nofuse=True,
                            )
                        )
                        n += 1
                    inst.sync_info = mybir.SyncInfo(
                        on_wait=[waits[-1]], on_update=list(si.on_update)
                    )
                    changed = True
                new.append(inst)
            if changed:
                bb.instructions[:] = new
    return n


def _build(**opts):
    key = ("nc", tuple(sorted(opts.items())))
    if key in _cache:
        return _cache[key]
    fp8 = opts.get("fp8", True)
    psa_bufs = opts.get("psa_bufs", 6)
    psb_bufs = opts.get("psb_bufs", 2)
    xq_bufs = opts.get("xq_bufs", 2)
    mid_bufs = opts.get("mid_bufs", 2)
    outp_bufs = opts.get("outp_bufs", 3)
    small_bufs = opts.get("small_bufs", 1)
    ph0tmp_bufs = opts.get("ph0tmp_bufs", 2)
    nch = opts.get("nch", NCH)
    skip_ph0 = opts.get("skip_ph0", False)
    sep_bcast = opts.get("sep_bcast", False)
    q2_dve = opts.get("q2_dve", True)
    x8_pool = opts.get("x8_pool", False)
    late_rsum = opts.get("late_rsum", False)

    import concourse.bass as bass
    import concourse.mybir as mybir
    import concourse.tile as tile

    f32 = mybir.dt.float32
    bf16 = mybir.dt.bfloat16
    f8 = mybir.dt.float8e4
    wdt = f8 if fp8 else bf16      # dtype of Q/K/Vo weights + init
    AF = mybir.ActivationFunctionType
    MUL = mybir.AluOpType.mult
    ADD = mybir.AluOpType.add
    DR = mybir.MatmulPerfMode.DoubleRow
    KSTEP = 2 if fp8 else 1        # k-tiles consumed per matmul
    # with x32-scaled weights, projections come out x32 and norms^2 x1024
    nrm_scale = (1.0 / (W8SCALE * W8SCALE)) if fp8 else 1.0
    exp_scale = (1.0 / (W8SCALE * W8SCALE)) if fp8 else 1.0

    nc = bass.Bass("TRN2", debug=False)

    xT_d = nc.dram_tensor("xT", [H, S], bf16, kind="ExternalInput")
    initT_d = nc.dram_tensor("initT", [H, M], wdt, kind="ExternalInput")
    wqT_d = nc.dram_tensor("wqT", [H, H], wdt, kind="ExternalInput")
    wkT_d = nc.dram_tensor("wkT", [H, H], wdt, kind="ExternalInput")
    wvoT_d = nc.dram_tensor("wvoT", [H, H], wdt, kind="ExternalInput")
    wo1T_d = nc.dram_tensor("wo1T", [H, H], bf16, kind="ExternalInput")
    bq_d = nc.dram_tensor("bqt", [P, KT], f32, kind="ExternalInput")
    bk_d = nc.dram_tensor("bkt", [P, KT], f32, kind="ExternalInput")
    bo_d = nc.dram_tensor("bot", [P, KT], f32, kind="ExternalInput")
    borow_d = nc.dram_tensor("borow", [1, H], bf16, kind="ExternalInput")
    outT_d = nc.dram_tensor("outT", [H, S], f32, kind="ExternalOutput")

    def mm_chain(ps, w_sb, rhs_sb, wslice, rslice):
        """Accumulate out = sum_kt w[:, kt, wslice].T @ rhs[:, kt, rslice];
        fp8 mode pairs k-tiles via DoubleRow."""
        for kt in range(0, KT, KSTEP):
            nc.tensor.matmul(
                ps[:],
                w_sb[:, kt:kt + KSTEP, wslice] if fp8 else w_sb[:, kt, wslice],
                rhs_sb[:, kt:kt + KSTEP, rslice] if fp8 else rhs_sb[:, kt, rslice],
                start=(kt == 0),
                stop=(kt + KSTEP == KT),
                perf_mode=DR if fp8 else None,
            )

    with tile.TileContext(nc) as tc:
        with (
            tc.tile_pool(name="const", bufs=1) as const,
            tc.tile_pool(name="ph0", bufs=1) as ph0,
            tc.tile_pool(name="ph0tmp", bufs=ph0tmp_bufs) as ph0tmp,
            tc.tile_pool(name="xq", bufs=xq_bufs) as xq,
            tc.tile_pool(name="mid", bufs=mid_bufs) as mid,
            tc.tile_pool(name="outp", bufs=outp_bufs) as outp,
            tc.tile_pool(name="small", bufs=small_bufs) as small,
            tc.tile_pool(name="psA", bufs=psa_bufs, space="PSUM") as psA,
            tc.tile_pool(name="psB", bufs=psb_bufs, space="PSUM") as psB,
        ):
            wq_sb = const.tile([P, KT, H], wdt, name="wq_sb")
            wo1_sb = const.tile([P, KT, H], bf16, name="wo1_sb")
            kn_sb = const.tile([P, KT, M], wdt, name="kn_sb")   # knT: [h' part, m]
            v2_sb = const.tile([P, KT, H], wdt, name="v2_sb")   # V2: [m part, h']
            ones_sb = const.tile([P, 1], bf16, name="ones_sb")
            onesr_sb = const.tile([P, 1], bf16, name="onesr_sb")  # 1/ASCALE
            onesrow_sb = const.tile([1, P], f32, name="onesrow_sb")
            onessc_sb = const.tile([1, SC], bf16, name="onessc_sb")
            borow_sb = const.tile([1, H], bf16, name="borow_sb")
            bq_sb = const.tile([P, KT], f32, name="bq_sb")
            bk_sb = const.tile([P, KT], f32, name="bk_sb")
            bo_sb = const.tile([P, KT], f32, name="bo_sb")

            nc.vector.memset(ones_sb[:], 1.0)
            nc.vector.memset(onesr_sb[:], (1.0 / ASCALE) if fp8 else 1.0)
            nc.vector.memset(onesrow_sb[:], 1.0)
            nc.vector.memset(onessc_sb[:], 1.0)
            nc.sync.dma_start(out=bq_sb[:], in_=bq_d.ap())
            nc.sync.dma_start(out=bk_sb[:], in_=bk_d.ap())
            nc.sync.dma_start(out=bo_sb[:], in_=bo_d.ap())
            nc.sync.dma_start(out=borow_sb[:], in_=borow_d.ap())
            for kt in range(KT):
                nc.sync.dma_start(out=wq_sb[:, kt, :], in_=wqT_d.ap()[kt * P:(kt + 1) * P, :])
                nc.sync.dma_start(out=wo1_sb[:, kt, :], in_=wo1T_d.ap()[kt * P:(kt + 1) * P, :])

            # ---------- phase 0: memory init (overlaps with early Q chunks) ----
            wk_sb = ph0.tile([P, KT, H], wdt, name="wk_sb")
            wvo_sb = ph0.tile([P, KT, H], wdt, name="wvo_sb")
            init_sb = ph0.tile([P, KT, M], wdt, name="init_sb")
            kraw_sb = ph0.tile([P, KT, SC], bf16, name="kraw_sb")
            for kt in range(KT):
                nc.sync.dma_start(out=wk_sb[:, kt, :], in_=wkT_d.ap()[kt * P:(kt + 1) * P, :])
                nc.sync.dma_start(out=wvo_sb[:, kt, :], in_=wvoT_d.ap()[kt * P:(kt + 1) * P, :])
                nc.sync.dma_start(out=init_sb[:, kt, :], in_=initT_d.ap()[kt * P:(kt + 1) * P, :])

            # memory keys + column norms (norm over h' = partition dim via PE
            # ones-reduce on squared tiles); kn = Kmem/|Kmem| (x32 in fp8 mode)
            for mh in range(0 if skip_ph0 else (M // SC)):
                ms = slice(mh * SC, (mh + 1) * SC)
                kn2_ps = psB.tile([1, SC], f32, name="kn2_ps", tag="acc")
                for ht in range(KT):
                    ps = psA.tile([P, SC], f32, name="kps", tag="mm")
                    mm_chain(ps, wk_sb, init_sb, slice(ht * P, (ht + 1) * P), ms)
                    nc.scalar.activation(
                        out=kraw_sb[:, ht, :], in_=ps[:], func=AF.Identity,
                        bias=bk_sb[:, ht:ht + 1],
                    )
                    k2 = ph0tmp.tile([P, SC], bf16, name="k2")
                    nc.scalar.activation(
                        out=k2[:], in_=ps[:], func=AF.Square,
                        bias=bk_sb[:, ht:ht + 1],
                    )
                    nc.tensor.matmul(
                        kn2_ps[:], ones_sb[:], k2[:],
                        start=(ht == 0), stop=(ht == KT - 1),
                    )
                knorm = ph0tmp.tile([1, SC], f32, name="knorm")
                nc.scalar.activation(out=knorm[:], in_=kn2_ps[:], func=AF.Sqrt,
                                     scale=nrm_scale)
                kscale = ph0tmp.tile([1, SC], f32, name="kscale")
                nc.vector.reciprocal(out=kscale[:], in_=knorm[:])
                kscale_b = (psB.tile([P, SC], f32, name="kscale_b", tag="bc") if sep_bcast
                            else psA.tile([P, SC], f32, name="kscale_b", tag="mm"))
                nc.tensor.matmul(kscale_b[:], onesrow_sb[:], kscale[:], start=True, stop=True)
                for ht in range(KT):
                    nc.vector.tensor_tensor(
                        out=kn_sb[:, ht, ms], in0=kraw_sb[:, ht, :],
                        in1=kscale_b[:], op=MUL,
                    )

            # V2[m, h'] = init @ Wvo^T  (Wvo = Wo2 @ Wv, host-precomputed)
            for mt in range(0 if skip_ph0 else MT):
                for hh in range(H // SC):
                    hs = slice(hh * SC, (hh + 1) * SC)
                    ps = psA.tile([P, SC], f32, name="v2ps", tag="mm")
                    mm_chain(ps, init_sb, wvo_sb, slice(mt * P, (mt + 1) * P), hs)
                    nc.scalar.activation(out=v2_sb[:, mt, hs], in_=ps[:], func=AF.Copy,
                                         scale=(1.0 / W8SCALE) if fp8 else 1.0)

            # ---------- phase 1: per s-chunk ----------
            for ch in range(nch):
                cs = slice(ch * SC, (ch + 1) * SC)
                x_sb = xq.tile([P, KT, SC], bf16, name="x_sb")
                for kt in range(KT):
                    nc.sync.dma_start(out=x_sb[:, kt, :], in_=xT_d.ap()[kt * P:(kt + 1) * P, cs])
                if fp8:
                    x8_sb = xq.tile([P, KT, SC], f8, name="x8_sb")
                    for kt in range(KT):
                        (nc.gpsimd if x8_pool else nc.vector).tensor_copy(
                            out=x8_sb[:, kt, :], in_=x_sb[:, kt, :])
                else:
                    x8_sb = x_sb

                # Q projection + column norms; qn = Q/|Q| (x32 in fp8 mode)
                qraw = xq.tile([P, KT, SC], bf16, name="qraw")
                qn = xq.tile([P, KT, SC], wdt, name="qn")
                qn2_ps = psB.tile([1, SC], f32, name="qn2_ps", tag="acc")
                for ht in range(KT):
                    ps = psA.tile([P, SC], f32, name="qps", tag="mm")
                    mm_chain(ps, wq_sb, x8_sb, slice(ht * P, (ht + 1) * P), slice(None))
                    nc.scalar.activation(
                        out=qraw[:, ht, :], in_=ps[:], func=AF.Identity,
                        bias=bq_sb[:, ht:ht + 1],
                    )
                    q2 = mid.tile([P, SC], bf16, name="q2")
                    if q2_dve:
                        nc.vector.tensor_tensor(
                            out=q2[:], in0=qraw[:, ht, :], in1=qraw[:, ht, :], op=MUL,
                        )
                    else:
                        nc.scalar.activation(
                            out=q2[:], in_=ps[:], func=AF.Square,
                            bias=bq_sb[:, ht:ht + 1],
                        )
                    nc.tensor.matmul(
                        qn2_ps[:], ones_sb[:], q2[:],
                        start=(ht == 0), stop=(ht == KT - 1),
                    )
                qnorm = small.tile([1, SC], f32, name="qnorm")
                nc.scalar.activation(out=qnorm[:], in_=qn2_ps[:], func=AF.Sqrt,
                                     scale=nrm_scale)
                nc.vector.reciprocal(out=qnorm[:], in_=qnorm[:])
                qscale_b = (psB.tile([P, SC], f32, name="qscale_b", tag="bc") if sep_bcast
                            else psA.tile([P, SC], f32, name="qscale_b", tag="mm"))
                nc.tensor.matmul(qscale_b[:], onesrow_sb[:], qnorm[:], start=True, stop=True)
                for ht in range(KT):
                    nc.vector.tensor_tensor(
                        out=qn[:, ht, :], in0=qraw[:, ht, :], in1=qscale_b[:], op=MUL,
                    )

                # cosine sims -> exp -> softmax weights
                if late_rsum and fp8:
                    # raw exp in fp8 feeds the V2 matmul; 1/sum(exp) applied
                    # at the final combine, off the critical path
                    expT = xq.tile([P, MT, SC], wdt, name="expT")
                    attnT = expT
                else:
                    expT = xq.tile([P, MT, SC], bf16, name="expT")
                    attnT = xq.tile([P, MT, SC], wdt, name="attnT") if fp8 else expT
                se_ps = psB.tile([1, SC], f32, name="se_ps", tag="acc")
                for mt in range(MT):
                    ps = psA.tile([P, SC], f32, name="sps", tag="mm")
                    mm_chain(ps, kn_sb, qn, slice(mt * P, (mt + 1) * P), slice(None))
                    nc.scalar.activation(out=expT[:, mt, :], in_=ps[:], func=AF.Exp,
                                         scale=exp_scale)
                    nc.tensor.matmul(
                        se_ps[:], onesr_sb[:], expT[:, mt, :],
                        start=(mt == 0), stop=(mt == MT - 1),
                    )
                rsum = small.tile([1, SC], f32, name="rsum")
                nc.vector.reciprocal(out=rsum[:], in_=se_ps[:])
                rsum_b = (psB.tile([P, SC], f32, name="rsum_b", tag="bc") if sep_bcast
                          else psA.tile([P, SC], f32, name="rsum_b", tag="mm"))
                nc.tensor.matmul(rsum_b[:], onesrow_sb[:], rsum[:], start=True, stop=True)
                if late_rsum and fp8:
                    # DVE can read only one PSUM operand; park the broadcast in SBUF
                    rsum_sb = small.tile([P, SC], f32, name="rsum_sb")
                    nc.scalar.activation(out=rsum_sb[:], in_=rsum_b[:], func=AF.Copy)
                if not (late_rsum and fp8):
                    for mt in range(MT):
                        nc.vector.tensor_tensor(
                            out=attnT[:, mt, :], in0=expT[:, mt, :], in1=rsum_b[:], op=MUL,
                        )

                # out = Wo1 @ xT (bf16 chain) + V2^T @ attnT (fp8 chain) + bo
                for ht in range(KT):
                    hsl = slice(ht * P, (ht + 1) * P)
                    psW = psA.tile([P, SC], f32, name="opsW", tag="mm")
                    wo1_stop = (fp8 and not late_rsum)
                    for kt in range(KT):
                        nc.tensor.matmul(
                            psW[:],
                            wo1_sb[:, kt, hsl],
                            x_sb[:, kt, :],
                            start=(kt == 0),
                            stop=(kt == KT - 1) if wo1_stop else False,
                        )
                    if late_rsum and fp8:
                        # bo lands here since the combine path has no bias slot
                        nc.tensor.matmul(
                            psW[:], borow_sb[:, hsl], onessc_sb[:],
                            start=False, stop=True,
                        )
                    if fp8:
                        psV = psA.tile([P, SC], f32, name="opsV", tag="mm")
                        mm_chain(psV, v2_sb, attnT, hsl, slice(None))
                        o_sb = outp.tile([P, SC], f32, name="o_sb")
                        if late_rsum:
                            # o = psV * (ASCALE/sum) / ASCALE + bo, then + psW
                            ot = outp.tile([P, SC], f32, name="ot")
                            nc.vector.tensor_tensor(
                                out=ot[:], in0=psV[:], in1=rsum_sb[:], op=MUL,
                            )
                            nc.vector.scalar_tensor_tensor(
                                out=o_sb[:], in0=ot[:], scalar=1.0 / ASCALE,
                                in1=psW[:], op0=MUL, op1=ADD,
                            )
                        else:
                            nc.scalar.activation(
                                out=o_sb[:], in_=psV[:], func=AF.Identity,
                                bias=bo_sb[:, ht:ht + 1], scale=1.0 / ASCALE,
                            )
                            nc.vector.tensor_tensor(
                                out=o_sb[:], in0=o_sb[:], in1=psW[:], op=ADD,
                            )
                    else:
                        for mt in range(MT):
                            nc.tensor.matmul(
                                psW[:],
                                v2_sb[:, mt, hsl],
                                attnT[:, mt, :],
                                start=False,
                                stop=(mt == MT - 1),
                            )
                        o_sb = outp.tile([P, SC], f32, name="o_sb")
                        nc.scalar.activation(
                            out=o_sb[:], in_=psW[:], func=AF.Identity,
                            bias=bo_sb[:, ht:ht + 1],
                        )
                    nc.sync.dma_start(out=outT_d.ap()[ht * P:(ht + 1) * P, cs], in_=o_sb[:])

    _split_excess_waits(nc, mybir)
    _cache[key] = nc
    return nc


def _prep_inputs(hidden_states, Wq, bq, Wk, bk, Wv, bv, Wo, bo, fp8=True):
    hidden_states = np.asarray(hidden_states, dtype=np.float32)
    idx = _idx()

    Wq = np.asarray(Wq, np.float32)
    Wk = np.asarray(Wk, np.float32)
    Wv = np.asarray(Wv, np.float32)
    Wo = np.asarray(Wo, np.float32)
    bq = np.asarray(bq, np.float32)
    bk = np.asarray(bk, np.float32)
    bv = np.asarray(bv, np.float32)
    bo = np.asarray(bo, np.float32)
    Wo2 = Wo[:, H:]
    Wvo = Wo2 @ Wv          # folds (attn @ Vmem) @ Wo2^T into attn @ V2
    bo_eff = bo + Wo2 @ bv  # softmax rows sum to 1 -> bv lands as a constant

    def cvt(a, scale=1.0):
        a = np.ascontiguousarray(a.T) * scale
        if fp8:
            return np.clip(a, -240.0, 240.0).astype(FP8)
        return a.astype(BF16)

    ws = W8SCALE if fp8 else 1.0
    wqT = cvt(Wq, ws)
    wkT = cvt(Wk, ws)
    wvoT = cvt(Wvo, ws)
    wo1T = np.ascontiguousarray(Wo[:, :H].T).astype(BF16)

    def btile(b):
        return np.ascontiguousarray(b.reshape(KT, P).T * ws)

    bqt, bkt = btile(bq), btile(bk)
    bot = np.ascontiguousarray(bo_eff.reshape(KT, P).T)

    in_maps = []
    for b in range(B):
        xT = np.ascontiguousarray(hidden_states[b].T).astype(BF16)
        init = xT[:, idx]
        in_maps.append({
            "xT": xT,
            "initT": np.ascontiguousarray(init).astype(FP8) if fp8
                     else np.ascontiguousarray(init),
            "wqT": wqT, "wkT": wkT, "wvoT": wvoT, "wo1T": wo1T,
            "bqt": bqt, "bkt": bkt, "bot": bot,
        })
    return in_maps


def kernel(hidden_states, Wq, bq, Wk, bk, Wv, bv, Wo, bo):
    from concourse import bass_utils

    in_maps = _prep_inputs(hidden_states, Wq, bq, Wk, bk, Wv, bv, Wo, bo)
    nc = _build()
    res = bass_utils.run_bass_kernel_spmd(nc, in_maps, core_ids=list(range(N_CORES)))

    out = np.empty((B, S, H), np.float32)
    for b in range(B):
        out[b] = res.results[b]["outT"].T
    return out
